# revision 60
# baseline (speedup 1.0000x reference)
"""Trainium2 Bass kernel for nn_Attention (B=32,T=512,S=2048,D=512), 8-core data parallel.

Per core: 4 batch elements. Per batch:
  x^T   = W_in^T.T @ input^T              (mm1, float32r)
  logits= x^T.T @ src^T                   (mm2, float32r)   [t,s]
  attn  = softmax(logits)  (exact, fp32)  -> DRAM output
  mix^T = src.T @ attn^T                  (mm3, bf16)
  out   = tanh([mix^T;input^T].T @ W_out^T) (mm4, bf16) -> DRAM output

Transposes on PE via identity-matmul into PSUM staging [128,4,128] groups,
copied out (with dtype rounding/cast) on DVE/ACT. fp32r operand chains are
typed float32r end-to-end (walrus requires producers to emit fp32r).
"""
import sys

sys.path.insert(0, "/opt/trn_rl_repo")

import numpy as np

import concourse.bass as bass  # noqa: F401
import concourse.tile as tile
from concourse import bacc, mybir
from concourse.masks import make_identity

F32 = mybir.dt.float32
F32R = mybir.dt.float32r
BF16 = mybir.dt.bfloat16
AX = mybir.AxisListType
AF = mybir.ActivationFunctionType


def build_nc(B=4, T=512, S=2048, D=512):
    TC, SC, DC = T // 128, S // 128, D // 128
    C = 2 * D
    CC = C // 128
    SG = 4            # source_hids DMA'd in SG groups
    SCG = SC // SG

    import concourse.tile_utils as tile_utils
    tile_utils.max_sbuf_usage = 206 * 1024

    nc = bacc.Bacc(None, target_bir_lowering=False, debug=False)
    inp_d = nc.declare_dram_parameter("input", [B, T, D], F32R, isOutput=False)
    src_d = nc.declare_dram_parameter("source_hids", [B, S, D], F32R, isOutput=False)
    win_d = nc.declare_dram_parameter("W_in", [D, D], F32R, isOutput=False)
    wout_d = nc.declare_dram_parameter("W_out", [D, C], F32, isOutput=False)
    out_d = nc.declare_dram_parameter("output", [B, T, D], F32, isOutput=True)
    attn_d = nc.declare_dram_parameter("attn", [B, T, S], F32, isOutput=True)

    from contextlib import ExitStack

    with ExitStack() as ctx:
        tc = ctx.enter_context(tile.TileContext(nc))
        if True:
            pool = lambda name, bufs, **kw: ctx.enter_context(
                tc.tile_pool(name=name, bufs=bufs, **kw)
            )
            consts = pool("consts", 1)
            wkeep = pool("wkeep", 1)
            inputp = pool("inputp", 2)
            inputTp = pool("inputTp", 1)
            inputTbp = pool("inputTbp", 2)
            srcgp = pool("srcgp", 3)
            srcbp = pool("srcbp", 2)
            srcTp = pool("srcTp", 1)
            xTp = pool("xTp", 2)
            attnp = pool("attnp", 3)
            attnTp = pool("attnTp", 1)
            mixTp = pool("mixTp", 2)
            outp = pool("outp", 3)
            smalls = pool("smalls", 4)
            logp = pool("logp", 4, space="PSUM")
            mmoutp = pool("mmoutp", 2, space="PSUM")
            tpsp = pool("tpsp", 2, space="PSUM")

            ident = consts.tile([128, 128], F32)
            make_identity(nc, ident)
            ident_r = consts.tile([128, 128], F32R)
            nc.vector.tensor_copy(ident_r, ident)
            def pe_transpose(dst_psum, src_sbuf, rounded):
                idn = ident_r if rounded else ident
                nc.tensor.matmul(dst_psum, src_sbuf, idn, is_transpose=True)

            copy_flip = [0]

            def copy_out(dst, src_psum, engine=None):
                if engine is None:
                    engine = "act" if copy_flip[0] % 2 == 0 else "dve"
                    copy_flip[0] += 1
                if engine == "act":
                    nc.scalar.copy(dst, src_psum)
                else:
                    nc.vector.tensor_copy(dst, src_psum)

            # ---- batch-0 loads first so they overlap weight prep ----
            def load_input(b):
                t = inputp.tile([128, TC, D], F32R, tag="inp")
                nc.sync.dma_start(
                    out=t, in_=inp_d[b].rearrange("(c p) d -> p c d", p=128)
                )
                return t

            def load_src_group(b, g):
                sg = srcgp.tile([128, SCG, D], F32R, tag="srcg")
                nc.sync.dma_start(
                    out=sg,
                    in_=src_d[b, g * SCG * 128:(g + 1) * SCG * 128, :].rearrange(
                        "(c p) d -> p c d", p=128
                    ),
                )
                return sg

            inp_next = load_input(0)

            # ---- weights (once; staged through the srcgp pool) ----
            # W_in [e,d] -> W_inT [d-in, DC, e]  (f32r, feeds mm1 lhsT)
            win_s = srcgp.tile([128, DC, D], F32R, tag="srcg")
            nc.scalar.dma_start(out=win_s, in_=win_d.rearrange("(c p) d -> p c d", p=128))
            winT = wkeep.tile([128, DC, D], F32R, tag="winT")
            for ec in range(DC):
                st = tpsp.tile([128, 4, 128], F32R, tag="tps")
                for dc in range(DC):
                    pe_transpose(st[:, dc, :], win_s[:, ec, dc * 128:(dc + 1) * 128], True)
                copy_out(winT[:, :, ec * 128:(ec + 1) * 128], st)
            # W_out [o,c] -> W_outT [c-in, CC, o] bf16, staged in two halves
            woutT = wkeep.tile([128, CC, D], BF16, tag="woutT")
            for h in range(2):
                wout_s = srcgp.tile([128, DC, D], F32, tag="srcg")
                nc.scalar.dma_start(
                    out=wout_s,
                    in_=wout_d[:, h * D:(h + 1) * D].rearrange("(c p) d -> p c d", p=128),
                )
                for oc in range(DC):
                    st = tpsp.tile([128, 4, 128], F32, tag="tps")
                    for q in range(4):
                        pe_transpose(st[:, q, :], wout_s[:, oc, q * 128:(q + 1) * 128], False)
                    copy_out(woutT[:, 4 * h:4 * h + 4, oc * 128:(oc + 1) * 128], st)

            state = {
                "inp_next": inp_next,
                "src_next": [load_src_group(0, 0), load_src_group(0, 1)],
            }

            def prep(b):
                """input^T + mm1 + src^T + casts for batch b. Emitted between
                mm2(b-1) and mm3(b-1) so the PE covers softmax tails."""
                inp_t = state["inp_next"]
                src_g = state["src_next"]

                inpT = inputTp.tile([128, DC, T], F32R, tag="inpT")
                for c in range(TC):
                    st = tpsp.tile([128, 4, 128], F32R, tag="tps")
                    for k in range(DC):
                        pe_transpose(st[:, k, :], inp_t[:, c, k * 128:(k + 1) * 128], True)
                    copy_out(inpT[:, :, c * 128:(c + 1) * 128], st)
                inpTb = inputTbp.tile([128, DC, T], BF16, tag="inpTb")
                nc.vector.tensor_copy(inpTb, inpT.bitcast(F32))
                if b + 1 < B:
                    state["inp_next"] = load_input(b + 1)

                xT = xTp.tile([128, DC, T], F32R, tag="xT")
                for m in range(DC):
                    ps = mmoutp.tile([128, T], F32, tag="mmout")
                    for k in range(DC):
                        nc.tensor.matmul(
                            ps,
                            winT[:, k, m * 128:(m + 1) * 128],
                            inpT[:, k, :],
                            start=(k == 0),
                            stop=(k == DC - 1),
                        )
                    copy_out(xT[:, m, :], ps)

                srcT = srcTp.tile([128, DC, S], F32R, tag="srcT")
                srcb = srcbp.tile([128, SC, D], BF16, tag="srcb")
                for g in range(SG):
                    sg = src_g.pop(0)
                    for jj in range(SCG):
                        j = g * SCG + jj
                        st = tpsp.tile([128, 4, 128], F32R, tag="tps")
                        for m in range(DC):
                            pe_transpose(
                                st[:, m, :], sg[:, jj, m * 128:(m + 1) * 128], True
                            )
                        copy_out(srcT[:, :, j * 128:(j + 1) * 128], st)
                    h = SCG // 2
                    nc.scalar.copy(
                        srcb[:, g * SCG:g * SCG + h, :], sg[:, :h, :].bitcast(F32)
                    )
                    nc.vector.tensor_copy(
                        srcb[:, g * SCG + h:(g + 1) * SCG, :], sg[:, h:, :].bitcast(F32)
                    )
                    nb, ng = (b, g + 2) if g + 2 < SG else (b + 1, g + 2 - SG)
                    if nb < B:
                        src_g.append(load_src_group(nb, ng))
                state["src_next"] = src_g
                return dict(inpT=inpT, inpTb=inpTb, xT=xT, srcT=srcT, srcb=srcb)

            P = prep(0)
            for b in range(B):
                cur = P
                xT, srcT, srcb, inpTb = cur["xT"], cur["srcT"], cur["srcb"], cur["inpTb"]

                # ---- per t-chunk: mm2 (fp32r) + softmax + attn out + attn^T ----
                # attn^T transposes for chunk i are emitted after mm2(i+1) so the
                # PE has work while softmax(i+1) drains the logits banks.
                attnT = attnTp.tile([128, SC, T], BF16, tag="attnT")
                attn_tiles = []

                def emit_attnT(i, attn_sb, quarters=range(4)):
                    for gg0 in quarters:
                        for gg in range(gg0 * (SC // 16), (gg0 + 1) * (SC // 16)):
                            st = tpsp.tile([128, 4, 128], F32R, tag="tps")
                            for q in range(4):
                                j = gg * 4 + q
                                pe_transpose(st[:, q, :], attn_sb[:, j * 128:(j + 1) * 128], True)
                            copy_out(attnT[:, gg * 4:gg * 4 + 4, i * 128:(i + 1) * 128], st)

                for i in range(TC):
                    partmax = smalls.tile([128, S // 512], F32, tag="partmax")
                    logits_j = []
                    for j in range(S // 512):
                        lg = logp.tile([128, 512], F32, tag="logits")
                        logits_j.append(lg)
                        for k in range(DC):
                            nc.tensor.matmul(
                                lg,
                                xT[:, k, i * 128:(i + 1) * 128],
                                srcT[:, k, j * 512:(j + 1) * 512],
                                start=(k == 0),
                                stop=(k == DC - 1),
                            )
                        nc.vector.tensor_reduce(
                            partmax[:, j:j + 1], lg, axis=AX.X, op=mybir.AluOpType.max
                        )
                        # interleave a quarter of chunk i-1's attn^T transposes
                        # after each j-chain: PE slack for the per-slot WARs
                        if i > 0:
                            emit_attnT(i - 1, attn_tiles[i - 1], quarters=[j])
                    neg_max = smalls.tile([128, 1], F32, tag="negmax")
                    nc.vector.tensor_reduce(
                        neg_max, partmax, axis=AX.X, op=mybir.AluOpType.max, negate=True
                    )
                    attn_sb = attnp.tile([128, S], F32R, tag="attn")
                    sumexp4 = smalls.tile([128, S // 512], F32, tag="sumexp")
                    # per-slice exp frees each logits PSUM slot as soon as it's read
                    for j in range(S // 512):
                        nc.scalar.activation(
                            attn_sb[:, j * 512:(j + 1) * 512],
                            logits_j[j],
                            AF.Exp,
                            bias=neg_max[:, 0:1],
                            accum_out=sumexp4[:, j:j + 1],
                        )
                    sumtot = smalls.tile([128, 1], F32, tag="sumtot")
                    nc.vector.tensor_reduce(
                        sumtot, sumexp4, axis=AX.X, op=mybir.AluOpType.add
                    )
                    inv = smalls.tile([128, 1], F32, tag="inv")
                    nc.vector.reciprocal(inv, sumtot)
                    nc.vector.tensor_scalar_mul(attn_sb, attn_sb, inv[:, 0:1])
                    nc.gpsimd.dma_start(
                        out=attn_d[b, i * 128:(i + 1) * 128, :],
                        in_=attn_sb.bitcast(F32),
                    )
                    attn_tiles.append(attn_sb)

                # next batch's prep covers the softmax/attnT tail on the PE
                if b + 1 < B:
                    P = prep(b + 1)
                emit_attnT(TC - 1, attn_tiles[TC - 1])

                # ---- mm3: mix^T[e,t] (bf16) ----
                mixT = mixTp.tile([128, DC, T], BF16, tag="mixT")
                for m in range(DC):
                    ps = mmoutp.tile([128, T], F32, tag="mmout")
                    for j in range(SC):
                        nc.tensor.matmul(
                            ps,
                            srcb[:, j, m * 128:(m + 1) * 128],
                            attnT[:, j, :],
                            start=(j == 0),
                            stop=(j == SC - 1),
                        )
                    copy_out(mixT[:, m, :], ps)

                # ---- mm4: out[t,o] = tanh(comb^T.T @ W_out^T) (bf16) ----
                for i in range(TC):
                    ps = mmoutp.tile([128, D], F32, tag="mmout")
                    for k in range(CC):
                        lhsT = (
                            mixT[:, k, i * 128:(i + 1) * 128]
                            if k < DC
                            else inpTb[:, k - DC, i * 128:(i + 1) * 128]
                        )
                        nc.tensor.matmul(
                            ps, lhsT, woutT[:, k, :], start=(k == 0), stop=(k == CC - 1)
                        )
                    ot = outp.tile([128, D], F32, tag="out")
                    nc.scalar.activation(ot, ps, AF.Tanh)
                    nc.gpsimd.dma_start(out=out_d[b, i * 128:(i + 1) * 128, :], in_=ot)

    nc.compile()
    return nc


_NC_CACHE = {}


def get_nc(B=4, T=512, S=2048, D=512):
    key = (B, T, S, D)
    if key not in _NC_CACHE:
        _NC_CACHE[key] = build_nc(B, T, S, D)
    return _NC_CACHE[key]


def kernel(input, source_hids, W_in, W_out, trace=False, trace_kwargs=None):
    from concourse.bass_utils import run_bass_kernel_spmd

    input = np.ascontiguousarray(input, dtype=np.float32)
    source_hids = np.ascontiguousarray(source_hids, dtype=np.float32)
    W_in = np.ascontiguousarray(W_in, dtype=np.float32)
    W_out = np.ascontiguousarray(W_out, dtype=np.float32)

    NCORE = 8
    Bfull = input.shape[0]
    bpc = Bfull // NCORE
    nc = get_nc(B=bpc, T=input.shape[1], S=source_hids.shape[1], D=input.shape[2])

    in_maps = [
        {
            "input": input[c * bpc:(c + 1) * bpc],
            "source_hids": source_hids[c * bpc:(c + 1) * bpc],
            "W_in": W_in,
            "W_out": W_out,
        }
        for c in range(NCORE)
    ]
    res = run_bass_kernel_spmd(
        nc, in_maps, core_ids=list(range(NCORE)), trace=trace, **(trace_kwargs or {})
    )
    output = np.concatenate([res.results[c]["output"] for c in range(NCORE)], axis=0)
    attn = np.concatenate([res.results[c]["attn"] for c in range(NCORE)], axis=0)
    if trace:
        kernel.last_exec_time_ns = res.exec_time_ns
        kernel.last_results = res
    return output, attn


# revision 62
# speedup vs baseline: 1.0154x; 1.0154x over previous
"""Trainium2 Bass kernel for nn_Attention (B=32,T=512,S=2048,D=512), 8-core data parallel.

Per core: 4 batch elements. Per batch:
  x^T   = W_in^T.T @ input^T              (mm1, float32r)
  logits= x^T.T @ src^T                   (mm2, float32r)   [t,s]
  attn  = softmax(logits)  (exact, fp32)  -> DRAM output
  mix^T = src.T @ attn^T                  (mm3, bf16)
  out   = tanh([mix^T;input^T].T @ W_out^T) (mm4, bf16) -> DRAM output

Transposes on PE via identity-matmul into PSUM staging [128,4,128] groups,
copied out (with dtype rounding/cast) on DVE/ACT. fp32r operand chains are
typed float32r end-to-end (walrus requires producers to emit fp32r).
"""
import sys

sys.path.insert(0, "/opt/trn_rl_repo")

import numpy as np

import concourse.bass as bass  # noqa: F401
import concourse.tile as tile
from concourse import bacc, mybir
from concourse.masks import make_identity

F32 = mybir.dt.float32
F32R = mybir.dt.float32r
BF16 = mybir.dt.bfloat16
AX = mybir.AxisListType
AF = mybir.ActivationFunctionType


def build_nc(B=4, T=512, S=2048, D=512):
    TC, SC, DC = T // 128, S // 128, D // 128
    C = 2 * D
    CC = C // 128
    SG = 4            # source_hids DMA'd in SG groups
    SCG = SC // SG

    import concourse.tile_utils as tile_utils
    tile_utils.max_sbuf_usage = 206 * 1024

    nc = bacc.Bacc(None, target_bir_lowering=False, debug=False)
    inp_d = nc.declare_dram_parameter("input", [B, T, D], F32R, isOutput=False)
    src_d = nc.declare_dram_parameter("source_hids", [B, S, D], F32R, isOutput=False)
    win_d = nc.declare_dram_parameter("W_in", [D, D], F32R, isOutput=False)
    wout_d = nc.declare_dram_parameter("W_out", [D, C], F32, isOutput=False)
    out_d = nc.declare_dram_parameter("output", [B, T, D], F32, isOutput=True)
    attn_d = nc.declare_dram_parameter("attn", [B, T, S], F32, isOutput=True)

    from contextlib import ExitStack

    with ExitStack() as ctx:
        tc = ctx.enter_context(tile.TileContext(nc))
        if True:
            pool = lambda name, bufs, **kw: ctx.enter_context(
                tc.tile_pool(name=name, bufs=bufs, **kw)
            )
            consts = pool("consts", 1)
            wkeep = pool("wkeep", 1)
            inputp = pool("inputp", 2)
            inputTp = pool("inputTp", 1)
            inputTbp = pool("inputTbp", 2)
            srcgp = pool("srcgp", 3)
            srcbp = pool("srcbp", 2)
            srcTp = pool("srcTp", 1)
            xTp = pool("xTp", 2)
            attnp = pool("attnp", 3)
            attnTp = pool("attnTp", 1)
            mixTp = pool("mixTp", 2)
            outp = pool("outp", 3)
            smalls = pool("smalls", 4)
            logp = pool("logp", 4, space="PSUM")
            mmoutp = pool("mmoutp", 2, space="PSUM")
            tpsp = pool("tpsp", 2, space="PSUM")

            ident = consts.tile([128, 128], F32)
            make_identity(nc, ident)
            ident_r = consts.tile([128, 128], F32R)
            nc.vector.tensor_copy(ident_r, ident)
            def pe_transpose(dst_psum, src_sbuf, rounded):
                idn = ident_r if rounded else ident
                nc.tensor.matmul(dst_psum, src_sbuf, idn, is_transpose=True)

            copy_flip = [0]

            def copy_out(dst, src_psum, engine=None):
                if engine is None:
                    engine = "act" if copy_flip[0] % 2 == 0 else "dve"
                    copy_flip[0] += 1
                if engine == "act":
                    nc.scalar.copy(dst, src_psum)
                else:
                    nc.vector.tensor_copy(dst, src_psum)

            # ---- batch-0 loads first so they overlap weight prep ----
            def load_input(b):
                t = inputp.tile([128, TC, D], F32R, tag="inp")
                nc.sync.dma_start(
                    out=t, in_=inp_d[b].rearrange("(c p) d -> p c d", p=128)
                )
                return t

            def load_src_group(b, g):
                sg = srcgp.tile([128, SCG, D], F32R, tag="srcg")
                nc.sync.dma_start(
                    out=sg,
                    in_=src_d[b, g * SCG * 128:(g + 1) * SCG * 128, :].rearrange(
                        "(c p) d -> p c d", p=128
                    ),
                )
                return sg

            inp_next = load_input(0)

            # ---- weights (once; staged through the srcgp pool) ----
            # W_in [e,d] -> W_inT [d-in, DC, e]  (f32r, feeds mm1 lhsT)
            win_s = srcgp.tile([128, DC, D], F32R, tag="srcg")
            nc.scalar.dma_start(out=win_s, in_=win_d.rearrange("(c p) d -> p c d", p=128))
            winT = wkeep.tile([128, DC, D], F32R, tag="winT")
            for ec in range(DC):
                st = tpsp.tile([128, 4, 128], F32R, tag="tps")
                for dc in range(DC):
                    pe_transpose(st[:, dc, :], win_s[:, ec, dc * 128:(dc + 1) * 128], True)
                copy_out(winT[:, :, ec * 128:(ec + 1) * 128], st)
            # W_out [o,c] -> W_outT [c-in, CC, o] bf16, staged in two halves
            woutT = wkeep.tile([128, CC, D], BF16, tag="woutT")
            for h in range(2):
                wout_s = srcgp.tile([128, DC, D], F32, tag="srcg")
                nc.scalar.dma_start(
                    out=wout_s,
                    in_=wout_d[:, h * D:(h + 1) * D].rearrange("(c p) d -> p c d", p=128),
                )
                for oc in range(DC):
                    st = tpsp.tile([128, 4, 128], F32, tag="tps")
                    for q in range(4):
                        pe_transpose(st[:, q, :], wout_s[:, oc, q * 128:(q + 1) * 128], False)
                    copy_out(woutT[:, 4 * h:4 * h + 4, oc * 128:(oc + 1) * 128], st)

            state = {
                "inp_next": inp_next,
                "src_next": [load_src_group(0, 0), load_src_group(0, 1)],
            }

            def prep(b):
                """input^T + mm1 + src^T + casts for batch b. Emitted between
                mm2(b-1) and mm3(b-1) so the PE covers softmax tails."""
                inp_t = state["inp_next"]
                src_g = state["src_next"]

                inpT = inputTp.tile([128, DC, T], F32R, tag="inpT")
                for c in range(TC):
                    st = tpsp.tile([128, 4, 128], F32R, tag="tps")
                    for k in range(DC):
                        pe_transpose(st[:, k, :], inp_t[:, c, k * 128:(k + 1) * 128], True)
                    copy_out(inpT[:, :, c * 128:(c + 1) * 128], st)
                inpTb = inputTbp.tile([128, DC, T], BF16, tag="inpTb")
                nc.vector.tensor_copy(inpTb, inpT.bitcast(F32))
                if b + 1 < B:
                    state["inp_next"] = load_input(b + 1)

                xT = xTp.tile([128, DC, T], F32R, tag="xT")
                for m in range(DC):
                    ps = mmoutp.tile([128, T], F32, tag="mmout")
                    for k in range(DC):
                        nc.tensor.matmul(
                            ps,
                            winT[:, k, m * 128:(m + 1) * 128],
                            inpT[:, k, :],
                            start=(k == 0),
                            stop=(k == DC - 1),
                        )
                    copy_out(xT[:, m, :], ps)

                srcT = srcTp.tile([128, DC, S], F32R, tag="srcT")
                srcb = srcbp.tile([128, SC, D], BF16, tag="srcb")
                for g in range(SG):
                    sg = src_g.pop(0)
                    for jj in range(SCG):
                        j = g * SCG + jj
                        st = tpsp.tile([128, 4, 128], F32R, tag="tps")
                        for m in range(DC):
                            pe_transpose(
                                st[:, m, :], sg[:, jj, m * 128:(m + 1) * 128], True
                            )
                        copy_out(srcT[:, :, j * 128:(j + 1) * 128], st)
                    h = SCG // 2
                    nc.scalar.copy(
                        srcb[:, g * SCG:g * SCG + h, :], sg[:, :h, :].bitcast(F32)
                    )
                    nc.vector.tensor_copy(
                        srcb[:, g * SCG + h:(g + 1) * SCG, :], sg[:, h:, :].bitcast(F32)
                    )
                    nb, ng = (b, g + 2) if g + 2 < SG else (b + 1, g + 2 - SG)
                    if nb < B:
                        src_g.append(load_src_group(nb, ng))
                state["src_next"] = src_g
                return dict(inpT=inpT, inpTb=inpTb, xT=xT, srcT=srcT, srcb=srcb)

            P = prep(0)
            for b in range(B):
                cur = P
                xT, srcT, srcb, inpTb = cur["xT"], cur["srcT"], cur["srcb"], cur["inpTb"]

                # ---- per t-chunk: mm2 (fp32r) + softmax + attn out + attn^T ----
                # attn^T transposes for chunk i are emitted after mm2(i+1) so the
                # PE has work while softmax(i+1) drains the logits banks.
                attnT = attnTp.tile([128, SC, T], BF16, tag="attnT")
                attn_tiles = []

                def emit_attnT(i, attn_sb, quarters=range(4)):
                    for gg0 in quarters:
                        for gg in range(gg0 * (SC // 16), (gg0 + 1) * (SC // 16)):
                            st = tpsp.tile([128, 4, 128], F32R, tag="tps")
                            for q in range(4):
                                j = gg * 4 + q
                                pe_transpose(st[:, q, :], attn_sb[:, j * 128:(j + 1) * 128], True)
                            copy_out(attnT[:, gg * 4:gg * 4 + 4, i * 128:(i + 1) * 128], st)

                for i in range(TC):
                    partmax = smalls.tile([128, S // 512], F32, tag="partmax")
                    logits_j = []
                    for j in range(S // 512):
                        lg = logp.tile([128, 512], F32, tag="logits")
                        logits_j.append(lg)
                        for k in range(DC):
                            nc.tensor.matmul(
                                lg,
                                xT[:, k, i * 128:(i + 1) * 128],
                                srcT[:, k, j * 512:(j + 1) * 512],
                                start=(k == 0),
                                stop=(k == DC - 1),
                            )
                        nc.vector.tensor_reduce(
                            partmax[:, j:j + 1], lg, axis=AX.X, op=mybir.AluOpType.max
                        )
                    neg_max = smalls.tile([128, 1], F32, tag="negmax")
                    nc.vector.tensor_reduce(
                        neg_max, partmax, axis=AX.X, op=mybir.AluOpType.max, negate=True
                    )
                    attn_sb = attnp.tile([128, S], F32R, tag="attn")
                    sumexp4 = smalls.tile([128, S // 512], F32, tag="sumexp")
                    # per-slice exp frees each logits PSUM slot as soon as it's read
                    for j in range(S // 512):
                        nc.scalar.activation(
                            attn_sb[:, j * 512:(j + 1) * 512],
                            logits_j[j],
                            AF.Exp,
                            bias=neg_max[:, 0:1],
                            accum_out=sumexp4[:, j:j + 1],
                        )
                    sumtot = smalls.tile([128, 1], F32, tag="sumtot")
                    nc.vector.tensor_reduce(
                        sumtot, sumexp4, axis=AX.X, op=mybir.AluOpType.add
                    )
                    inv = smalls.tile([128, 1], F32, tag="inv")
                    nc.vector.reciprocal(inv, sumtot)
                    nc.vector.tensor_scalar_mul(attn_sb, attn_sb, inv[:, 0:1])
                    nc.gpsimd.dma_start(
                        out=attn_d[b, i * 128:(i + 1) * 128, :],
                        in_=attn_sb.bitcast(F32),
                    )
                    attn_tiles.append(attn_sb)
                    if i > 0:
                        emit_attnT(i - 1, attn_tiles[i - 1])

                # next batch's prep covers the softmax/attnT tail on the PE
                if b + 1 < B:
                    P = prep(b + 1)
                emit_attnT(TC - 1, attn_tiles[TC - 1])

                # ---- mm3: mix^T[e,t] (bf16) ----
                mixT = mixTp.tile([128, DC, T], BF16, tag="mixT")
                for m in range(DC):
                    ps = mmoutp.tile([128, T], F32, tag="mmout")
                    for j in range(SC):
                        nc.tensor.matmul(
                            ps,
                            srcb[:, j, m * 128:(m + 1) * 128],
                            attnT[:, j, :],
                            start=(j == 0),
                            stop=(j == SC - 1),
                        )
                    copy_out(mixT[:, m, :], ps)

                # ---- mm4: out[t,o] = tanh(comb^T.T @ W_out^T) (bf16) ----
                for i in range(TC):
                    ps = mmoutp.tile([128, D], F32, tag="mmout")
                    for k in range(CC):
                        lhsT = (
                            mixT[:, k, i * 128:(i + 1) * 128]
                            if k < DC
                            else inpTb[:, k - DC, i * 128:(i + 1) * 128]
                        )
                        nc.tensor.matmul(
                            ps, lhsT, woutT[:, k, :], start=(k == 0), stop=(k == CC - 1)
                        )
                    ot = outp.tile([128, D], F32, tag="out")
                    nc.scalar.activation(ot, ps, AF.Tanh)
                    nc.gpsimd.dma_start(out=out_d[b, i * 128:(i + 1) * 128, :], in_=ot)

    nc.compile()
    return nc


_NC_CACHE = {}


def get_nc(B=4, T=512, S=2048, D=512):
    key = (B, T, S, D)
    if key not in _NC_CACHE:
        _NC_CACHE[key] = build_nc(B, T, S, D)
    return _NC_CACHE[key]


def kernel(input, source_hids, W_in, W_out, trace=False, trace_kwargs=None):
    from concourse.bass_utils import run_bass_kernel_spmd

    input = np.ascontiguousarray(input, dtype=np.float32)
    source_hids = np.ascontiguousarray(source_hids, dtype=np.float32)
    W_in = np.ascontiguousarray(W_in, dtype=np.float32)
    W_out = np.ascontiguousarray(W_out, dtype=np.float32)

    NCORE = 8
    Bfull = input.shape[0]
    bpc = Bfull // NCORE
    nc = get_nc(B=bpc, T=input.shape[1], S=source_hids.shape[1], D=input.shape[2])

    in_maps = [
        {
            "input": input[c * bpc:(c + 1) * bpc],
            "source_hids": source_hids[c * bpc:(c + 1) * bpc],
            "W_in": W_in,
            "W_out": W_out,
        }
        for c in range(NCORE)
    ]
    res = run_bass_kernel_spmd(
        nc, in_maps, core_ids=list(range(NCORE)), trace=trace, **(trace_kwargs or {})
    )
    output = np.concatenate([res.results[c]["output"] for c in range(NCORE)], axis=0)
    attn = np.concatenate([res.results[c]["attn"] for c in range(NCORE)], axis=0)
    if trace:
        kernel.last_exec_time_ns = res.exec_time_ns
        kernel.last_results = res
    return output, attn


# revision 64
# speedup vs baseline: 1.0712x; 1.0550x over previous
"""Trainium2 Bass kernel for nn_Attention (B=32,T=512,S=2048,D=512), 8-core data parallel.

Per core: 4 batch elements. Per batch:
  x^T   = W_in^T.T @ input^T              (mm1, float32r)
  logits= x^T.T @ src^T                   (mm2, float32r)   [t,s]
  attn  = softmax(logits)  (exact, fp32)  -> DRAM output
  mix^T = src.T @ attn^T                  (mm3, bf16)
  out   = tanh([mix^T;input^T].T @ W_out^T) (mm4, bf16) -> DRAM output

Transposes on PE via identity-matmul into PSUM staging [128,4,128] groups,
copied out (with dtype rounding/cast) on DVE/ACT. fp32r operand chains are
typed float32r end-to-end (walrus requires producers to emit fp32r).
"""
import sys

sys.path.insert(0, "/opt/trn_rl_repo")

import numpy as np

import concourse.bass as bass  # noqa: F401
import concourse.tile as tile
from concourse import bacc, mybir
from concourse.masks import make_identity

F32 = mybir.dt.float32
F32R = mybir.dt.float32r
BF16 = mybir.dt.bfloat16
AX = mybir.AxisListType
AF = mybir.ActivationFunctionType


def build_nc(B=4, T=512, S=2048, D=512):
    TC, SC, DC = T // 128, S // 128, D // 128
    C = 2 * D
    CC = C // 128
    SG = 4            # source_hids DMA'd in SG groups
    SCG = SC // SG

    import concourse.tile_utils as tile_utils
    tile_utils.max_sbuf_usage = 206 * 1024

    nc = bacc.Bacc(None, target_bir_lowering=False, debug=False)
    inp_d = nc.declare_dram_parameter("input", [B, T, D], F32R, isOutput=False)
    src_d = nc.declare_dram_parameter("source_hids", [B, S, D], F32R, isOutput=False)
    win_d = nc.declare_dram_parameter("W_in", [D, D], F32R, isOutput=False)
    wout_d = nc.declare_dram_parameter("W_out", [D, C], F32, isOutput=False)
    out_d = nc.declare_dram_parameter("output", [B, T, D], F32, isOutput=True)
    attn_d = nc.declare_dram_parameter("attn", [B, T, S], F32, isOutput=True)

    from contextlib import ExitStack

    with ExitStack() as ctx:
        tc = ctx.enter_context(tile.TileContext(nc))
        if True:
            pool = lambda name, bufs, **kw: ctx.enter_context(
                tc.tile_pool(name=name, bufs=bufs, **kw)
            )
            consts = pool("consts", 1)
            wkeep = pool("wkeep", 1)
            inputp = pool("inputp", 2)
            inputTp = pool("inputTp", 1)
            inputTbp = pool("inputTbp", 2)
            srcgp = pool("srcgp", 3)
            srcbp = pool("srcbp", 2)
            srcTp = pool("srcTp", 1)
            xTp = pool("xTp", 2)
            attnp = pool("attnp", 3)
            attnTp = pool("attnTp", 1)
            mixTp = pool("mixTp", 2)
            outp = pool("outp", 3)
            smalls = pool("smalls", 3)
            logp = pool("logp", 4, space="PSUM")
            mmoutp = pool("mmoutp", 2, space="PSUM")
            tpsp = pool("tpsp", 2, space="PSUM")

            ident = consts.tile([128, 128], F32)
            make_identity(nc, ident)
            ident_r = consts.tile([128, 128], F32R)
            nc.vector.tensor_copy(ident_r, ident)
            def pe_transpose(dst_psum, src_sbuf, rounded):
                idn = ident_r if rounded else ident
                nc.tensor.matmul(dst_psum, src_sbuf, idn, is_transpose=True)

            copy_flip = [0]

            def copy_out(dst, src_psum, engine=None):
                if engine is None:
                    engine = "act" if copy_flip[0] % 2 == 0 else "dve"
                    copy_flip[0] += 1
                if engine == "act":
                    nc.scalar.copy(dst, src_psum)
                else:
                    nc.vector.tensor_copy(dst, src_psum)

            # ---- batch-0 loads first so they overlap weight prep ----
            def load_input(b):
                t = inputp.tile([128, TC, D], F32R, tag="inp")
                nc.sync.dma_start(
                    out=t, in_=inp_d[b].rearrange("(c p) d -> p c d", p=128)
                )
                return t

            def load_src_group(b, g):
                sg = srcgp.tile([128, SCG, D], F32R, tag="srcg")
                nc.sync.dma_start(
                    out=sg,
                    in_=src_d[b, g * SCG * 128:(g + 1) * SCG * 128, :].rearrange(
                        "(c p) d -> p c d", p=128
                    ),
                )
                return sg

            inp_next = load_input(0)

            # ---- weights (once; staged through the srcgp pool) ----
            # W_in [e,d] -> W_inT [d-in, DC, e]  (f32r, feeds mm1 lhsT)
            win_s = srcgp.tile([128, DC, D], F32R, tag="srcg")
            nc.scalar.dma_start(out=win_s, in_=win_d.rearrange("(c p) d -> p c d", p=128))
            winT = wkeep.tile([128, DC, D], F32R, tag="winT")
            for ec in range(DC):
                st = tpsp.tile([128, 4, 128], F32R, tag="tps")
                for dc in range(DC):
                    pe_transpose(st[:, dc, :], win_s[:, ec, dc * 128:(dc + 1) * 128], True)
                copy_out(winT[:, :, ec * 128:(ec + 1) * 128], st)
            # W_out [o,c] -> W_outT [c-in, CC, o] bf16, staged in two halves
            woutT = wkeep.tile([128, CC, D], BF16, tag="woutT")
            for h in range(2):
                wout_s = srcgp.tile([128, DC, D], F32, tag="srcg")
                nc.scalar.dma_start(
                    out=wout_s,
                    in_=wout_d[:, h * D:(h + 1) * D].rearrange("(c p) d -> p c d", p=128),
                )
                for oc in range(DC):
                    st = tpsp.tile([128, 4, 128], F32, tag="tps")
                    for q in range(4):
                        pe_transpose(st[:, q, :], wout_s[:, oc, q * 128:(q + 1) * 128], False)
                    copy_out(woutT[:, 4 * h:4 * h + 4, oc * 128:(oc + 1) * 128], st)

            state = {
                "inp_next": inp_next,
                "src_next": [load_src_group(0, 0), load_src_group(0, 1)],
            }

            def prep(b):
                """input^T + mm1 + src^T + casts for batch b. Emitted between
                mm2(b-1) and mm3(b-1) so the PE covers softmax tails."""
                inp_t = state["inp_next"]
                src_g = state["src_next"]

                inpT = inputTp.tile([128, DC, T], F32R, tag="inpT")
                for c in range(TC):
                    st = tpsp.tile([128, 4, 128], F32R, tag="tps")
                    for k in range(DC):
                        pe_transpose(st[:, k, :], inp_t[:, c, k * 128:(k + 1) * 128], True)
                    copy_out(inpT[:, :, c * 128:(c + 1) * 128], st)
                inpTb = inputTbp.tile([128, DC, T], BF16, tag="inpTb")
                nc.vector.tensor_copy(inpTb, inpT.bitcast(F32))
                if b + 1 < B:
                    state["inp_next"] = load_input(b + 1)

                xT = xTp.tile([128, DC, T], F32R, tag="xT")
                for m in range(DC):
                    ps = mmoutp.tile([128, T], F32, tag="mmout")
                    for k in range(DC):
                        nc.tensor.matmul(
                            ps,
                            winT[:, k, m * 128:(m + 1) * 128],
                            inpT[:, k, :],
                            start=(k == 0),
                            stop=(k == DC - 1),
                        )
                    copy_out(xT[:, m, :], ps)

                srcT = srcTp.tile([128, DC, S], F32R, tag="srcT")
                srcb = srcbp.tile([128, SC, D], BF16, tag="srcb")
                for g in range(SG):
                    sg = src_g.pop(0)
                    for jj in range(SCG):
                        j = g * SCG + jj
                        st = tpsp.tile([128, 4, 128], F32R, tag="tps")
                        for m in range(DC):
                            pe_transpose(
                                st[:, m, :], sg[:, jj, m * 128:(m + 1) * 128], True
                            )
                        copy_out(srcT[:, :, j * 128:(j + 1) * 128], st)
                    h = SCG // 2
                    nc.scalar.copy(
                        srcb[:, g * SCG:g * SCG + h, :], sg[:, :h, :].bitcast(F32)
                    )
                    nc.vector.tensor_copy(
                        srcb[:, g * SCG + h:(g + 1) * SCG, :], sg[:, h:, :].bitcast(F32)
                    )
                    nb, ng = (b, g + 2) if g + 2 < SG else (b + 1, g + 2 - SG)
                    if nb < B:
                        src_g.append(load_src_group(nb, ng))
                state["src_next"] = src_g
                return dict(inpT=inpT, inpTb=inpTb, xT=xT, srcT=srcT, srcb=srcb)

            P = prep(0)
            for b in range(B):
                cur = P
                xT, srcT, srcb, inpTb = cur["xT"], cur["srcT"], cur["srcb"], cur["inpTb"]

                # ---- per t-chunk: mm2 (fp32r) + softmax + attn out + attn^T ----
                # attn^T transposes for chunk i are emitted after mm2(i+1) so the
                # PE has work while softmax(i+1) drains the logits banks.
                attnT = attnTp.tile([128, SC, T], BF16, tag="attnT")
                attn_tiles = []

                def emit_attnT(i, attn_sb, quarters=range(4)):
                    for gg0 in quarters:
                        for gg in range(gg0 * (SC // 16), (gg0 + 1) * (SC // 16)):
                            st = tpsp.tile([128, 4, 128], F32R, tag="tps")
                            for q in range(4):
                                j = gg * 4 + q
                                pe_transpose(st[:, q, :], attn_sb[:, j * 128:(j + 1) * 128], True)
                            copy_out(attnT[:, gg * 4:gg * 4 + 4, i * 128:(i + 1) * 128], st)

                NJ = S // 512
                for i in range(TC):
                    # flash-style: each slice exps against ITS OWN max so the
                    # logits slot frees immediately after its chain; the
                    # exp(m_j - M)/Z corrections fold into the final scale.
                    nm = smalls.tile([128, NJ], F32, tag="negpm")  # -m_j
                    sumexp4 = smalls.tile([128, NJ], F32, tag="sumexp")
                    attn_sb = attnp.tile([128, S], F32R, tag="attn")
                    for j in range(NJ):
                        lg = logp.tile([128, 512], F32, tag="logits")
                        for k in range(DC):
                            nc.tensor.matmul(
                                lg,
                                xT[:, k, i * 128:(i + 1) * 128],
                                srcT[:, k, j * 512:(j + 1) * 512],
                                start=(k == 0),
                                stop=(k == DC - 1),
                            )
                        nc.vector.tensor_reduce(
                            nm[:, j:j + 1], lg, axis=AX.X,
                            op=mybir.AluOpType.max, negate=True,
                        )
                        nc.scalar.activation(
                            attn_sb[:, j * 512:(j + 1) * 512],
                            lg,
                            AF.Exp,
                            bias=nm[:, j:j + 1],
                            accum_out=sumexp4[:, j:j + 1],
                        )
                    # NM = -M = min_j nm_j; dl_j = nm_j - NM = M - m_j >= 0
                    NM = smalls.tile([128, 1], F32, tag="NM")
                    nc.vector.tensor_reduce(NM, nm, axis=AX.X, op=mybir.AluOpType.min)
                    dl = smalls.tile([128, NJ], F32, tag="dl")
                    nc.vector.tensor_scalar_sub(dl, nm, NM[:, 0:1])
                    fac = smalls.tile([128, NJ], F32, tag="fac")
                    nc.scalar.activation(fac, dl, AF.Exp, scale=-1.0)  # exp(m_j - M)
                    wsum = smalls.tile([128, NJ], F32, tag="wsum")
                    nc.vector.tensor_mul(wsum, sumexp4, fac)
                    sumtot = smalls.tile([128, 1], F32, tag="sumtot")
                    nc.vector.tensor_reduce(
                        sumtot, wsum, axis=AX.X, op=mybir.AluOpType.add
                    )
                    inv = smalls.tile([128, 1], F32, tag="inv")
                    nc.vector.reciprocal(inv, sumtot)
                    ff = smalls.tile([128, NJ], F32, tag="ff")
                    nc.vector.tensor_scalar_mul(ff, fac, inv[:, 0:1])
                    for j in range(NJ):
                        nc.vector.tensor_scalar_mul(
                            attn_sb[:, j * 512:(j + 1) * 512],
                            attn_sb[:, j * 512:(j + 1) * 512],
                            ff[:, j:j + 1],
                        )
                    nc.gpsimd.dma_start(
                        out=attn_d[b, i * 128:(i + 1) * 128, :],
                        in_=attn_sb.bitcast(F32),
                    )
                    attn_tiles.append(attn_sb)
                    if i > 0:
                        emit_attnT(i - 1, attn_tiles[i - 1])

                # next batch's prep covers the softmax/attnT tail on the PE
                if b + 1 < B:
                    P = prep(b + 1)
                emit_attnT(TC - 1, attn_tiles[TC - 1])

                # ---- mm3: mix^T[e,t] (bf16) ----
                mixT = mixTp.tile([128, DC, T], BF16, tag="mixT")
                for m in range(DC):
                    ps = mmoutp.tile([128, T], F32, tag="mmout")
                    for j in range(SC):
                        nc.tensor.matmul(
                            ps,
                            srcb[:, j, m * 128:(m + 1) * 128],
                            attnT[:, j, :],
                            start=(j == 0),
                            stop=(j == SC - 1),
                        )
                    copy_out(mixT[:, m, :], ps)

                # ---- mm4: out[t,o] = tanh(comb^T.T @ W_out^T) (bf16) ----
                for i in range(TC):
                    ps = mmoutp.tile([128, D], F32, tag="mmout")
                    for k in range(CC):
                        lhsT = (
                            mixT[:, k, i * 128:(i + 1) * 128]
                            if k < DC
                            else inpTb[:, k - DC, i * 128:(i + 1) * 128]
                        )
                        nc.tensor.matmul(
                            ps, lhsT, woutT[:, k, :], start=(k == 0), stop=(k == CC - 1)
                        )
                    ot = outp.tile([128, D], F32, tag="out")
                    nc.scalar.activation(ot, ps, AF.Tanh)
                    nc.gpsimd.dma_start(out=out_d[b, i * 128:(i + 1) * 128, :], in_=ot)

    nc.compile()
    return nc


_NC_CACHE = {}


def get_nc(B=4, T=512, S=2048, D=512):
    key = (B, T, S, D)
    if key not in _NC_CACHE:
        _NC_CACHE[key] = build_nc(B, T, S, D)
    return _NC_CACHE[key]


def kernel(input, source_hids, W_in, W_out, trace=False, trace_kwargs=None):
    from concourse.bass_utils import run_bass_kernel_spmd

    input = np.ascontiguousarray(input, dtype=np.float32)
    source_hids = np.ascontiguousarray(source_hids, dtype=np.float32)
    W_in = np.ascontiguousarray(W_in, dtype=np.float32)
    W_out = np.ascontiguousarray(W_out, dtype=np.float32)

    NCORE = 8
    Bfull = input.shape[0]
    bpc = Bfull // NCORE
    nc = get_nc(B=bpc, T=input.shape[1], S=source_hids.shape[1], D=input.shape[2])

    in_maps = [
        {
            "input": input[c * bpc:(c + 1) * bpc],
            "source_hids": source_hids[c * bpc:(c + 1) * bpc],
            "W_in": W_in,
            "W_out": W_out,
        }
        for c in range(NCORE)
    ]
    res = run_bass_kernel_spmd(
        nc, in_maps, core_ids=list(range(NCORE)), trace=trace, **(trace_kwargs or {})
    )
    output = np.concatenate([res.results[c]["output"] for c in range(NCORE)], axis=0)
    attn = np.concatenate([res.results[c]["attn"] for c in range(NCORE)], axis=0)
    if trace:
        kernel.last_exec_time_ns = res.exec_time_ns
        kernel.last_results = res
    return output, attn


# revision 65
# speedup vs baseline: 1.0869x; 1.0146x over previous
"""Trainium2 Bass kernel for nn_Attention (B=32,T=512,S=2048,D=512), 8-core data parallel.

Per core: 4 batch elements. Per batch:
  x^T   = W_in^T.T @ input^T              (mm1, float32r)
  logits= x^T.T @ src^T                   (mm2, float32r)   [t,s]
  attn  = softmax(logits)  (exact, fp32)  -> DRAM output
  mix^T = src.T @ attn^T                  (mm3, bf16)
  out   = tanh([mix^T;input^T].T @ W_out^T) (mm4, bf16) -> DRAM output

Transposes on PE via identity-matmul into PSUM staging [128,4,128] groups,
copied out (with dtype rounding/cast) on DVE/ACT. fp32r operand chains are
typed float32r end-to-end (walrus requires producers to emit fp32r).
"""
import sys

sys.path.insert(0, "/opt/trn_rl_repo")

import numpy as np

import concourse.bass as bass  # noqa: F401
import concourse.tile as tile
from concourse import bacc, mybir
from concourse.masks import make_identity

F32 = mybir.dt.float32
F32R = mybir.dt.float32r
BF16 = mybir.dt.bfloat16
AX = mybir.AxisListType
AF = mybir.ActivationFunctionType


def build_nc(B=4, T=512, S=2048, D=512):
    TC, SC, DC = T // 128, S // 128, D // 128
    C = 2 * D
    CC = C // 128
    SG = 4            # source_hids DMA'd in SG groups
    SCG = SC // SG

    import concourse.tile_utils as tile_utils
    tile_utils.max_sbuf_usage = 206 * 1024

    nc = bacc.Bacc(None, target_bir_lowering=False, debug=False)
    inp_d = nc.declare_dram_parameter("input", [B, T, D], F32R, isOutput=False)
    src_d = nc.declare_dram_parameter("source_hids", [B, S, D], F32R, isOutput=False)
    win_d = nc.declare_dram_parameter("W_in", [D, D], F32R, isOutput=False)
    wout_d = nc.declare_dram_parameter("W_out", [D, C], F32, isOutput=False)
    out_d = nc.declare_dram_parameter("output", [B, T, D], F32, isOutput=True)
    attn_d = nc.declare_dram_parameter("attn", [B, T, S], F32, isOutput=True)

    from contextlib import ExitStack

    with ExitStack() as ctx:
        tc = ctx.enter_context(tile.TileContext(nc))
        if True:
            pool = lambda name, bufs, **kw: ctx.enter_context(
                tc.tile_pool(name=name, bufs=bufs, **kw)
            )
            consts = pool("consts", 1)
            wkeep = pool("wkeep", 1)
            inputp = pool("inputp", 2)
            inputTp = pool("inputTp", 1)
            inputTbp = pool("inputTbp", 2)
            srcgp = pool("srcgp", 3)
            srcbp = pool("srcbp", 2)
            srcTp = pool("srcTp", 1)
            xTp = pool("xTp", 2)
            attnp = pool("attnp", 3)
            attnTp = pool("attnTp", 1)
            mixTp = pool("mixTp", 2)
            outp = pool("outp", 3)
            smalls = pool("smalls", 3)
            logp = pool("logp", 4, space="PSUM")
            mmoutp = pool("mmoutp", 2, space="PSUM")
            tpsp = pool("tpsp", 2, space="PSUM")

            ident = consts.tile([128, 128], F32)
            make_identity(nc, ident)
            ident_r = consts.tile([128, 128], F32R)
            nc.vector.tensor_copy(ident_r, ident)
            def pe_transpose(dst_psum, src_sbuf, rounded):
                idn = ident_r if rounded else ident
                nc.tensor.matmul(dst_psum, src_sbuf, idn, is_transpose=True)

            copy_flip = [0]

            def copy_out(dst, src_psum, engine=None):
                if engine is None:
                    engine = "act" if copy_flip[0] % 2 == 0 else "dve"
                    copy_flip[0] += 1
                if engine == "act":
                    nc.scalar.copy(dst, src_psum)
                else:
                    nc.vector.tensor_copy(dst, src_psum)

            # ---- batch-0 loads first so they overlap weight prep ----
            def load_input(b):
                t = inputp.tile([128, TC, D], F32R, tag="inp")
                nc.sync.dma_start(
                    out=t, in_=inp_d[b].rearrange("(c p) d -> p c d", p=128)
                )
                return t

            def load_src_group(b, g):
                sg = srcgp.tile([128, SCG, D], F32R, tag="srcg")
                nc.sync.dma_start(
                    out=sg,
                    in_=src_d[b, g * SCG * 128:(g + 1) * SCG * 128, :].rearrange(
                        "(c p) d -> p c d", p=128
                    ),
                )
                return sg

            inp_next = load_input(0)

            # ---- weights (once; staged through the srcgp pool) ----
            # W_in [e,d] -> W_inT [d-in, DC, e]  (f32r, feeds mm1 lhsT)
            win_s = srcgp.tile([128, DC, D], F32R, tag="srcg")
            nc.scalar.dma_start(out=win_s, in_=win_d.rearrange("(c p) d -> p c d", p=128))
            winT = wkeep.tile([128, DC, D], F32R, tag="winT")
            for ec in range(DC):
                st = tpsp.tile([128, 4, 128], F32R, tag="tps")
                for dc in range(DC):
                    pe_transpose(st[:, dc, :], win_s[:, ec, dc * 128:(dc + 1) * 128], True)
                copy_out(winT[:, :, ec * 128:(ec + 1) * 128], st)
            # W_out [o,c] -> W_outT [c-in, CC, o] bf16, staged in two halves
            woutT = wkeep.tile([128, CC, D], BF16, tag="woutT")
            for h in range(2):
                wout_s = srcgp.tile([128, DC, D], F32, tag="srcg")
                nc.scalar.dma_start(
                    out=wout_s,
                    in_=wout_d[:, h * D:(h + 1) * D].rearrange("(c p) d -> p c d", p=128),
                )
                for oc in range(DC):
                    st = tpsp.tile([128, 4, 128], F32, tag="tps")
                    for q in range(4):
                        pe_transpose(st[:, q, :], wout_s[:, oc, q * 128:(q + 1) * 128], False)
                    copy_out(woutT[:, 4 * h:4 * h + 4, oc * 128:(oc + 1) * 128], st)

            state = {
                "inp_next": inp_next,
                "src_next": [load_src_group(0, 0), load_src_group(0, 1)],
            }

            def prep(b):
                """input^T + mm1 + src^T + casts for batch b. Emitted between
                mm2(b-1) and mm3(b-1) so the PE covers softmax tails."""
                inp_t = state["inp_next"]
                src_g = state["src_next"]

                inpT = inputTp.tile([128, DC, T], F32R, tag="inpT")
                for c in range(TC):
                    st = tpsp.tile([128, 4, 128], F32R, tag="tps")
                    for k in range(DC):
                        pe_transpose(st[:, k, :], inp_t[:, c, k * 128:(k + 1) * 128], True)
                    copy_out(inpT[:, :, c * 128:(c + 1) * 128], st)
                inpTb = inputTbp.tile([128, DC, T], BF16, tag="inpTb")
                nc.vector.tensor_copy(inpTb, inpT.bitcast(F32))
                if b + 1 < B:
                    state["inp_next"] = load_input(b + 1)

                xT = xTp.tile([128, DC, T], F32R, tag="xT")
                for m in range(DC):
                    ps = mmoutp.tile([128, T], F32, tag="mmout")
                    for k in range(DC):
                        nc.tensor.matmul(
                            ps,
                            winT[:, k, m * 128:(m + 1) * 128],
                            inpT[:, k, :],
                            start=(k == 0),
                            stop=(k == DC - 1),
                        )
                    copy_out(xT[:, m, :], ps)

                srcT = srcTp.tile([128, DC, S], F32R, tag="srcT")
                srcb = srcbp.tile([128, SC, D], BF16, tag="srcb")
                for g in range(SG):
                    sg = src_g.pop(0)
                    for jj in range(SCG):
                        j = g * SCG + jj
                        st = tpsp.tile([128, 4, 128], F32R, tag="tps")
                        for m in range(DC):
                            pe_transpose(
                                st[:, m, :], sg[:, jj, m * 128:(m + 1) * 128], True
                            )
                        copy_out(srcT[:, :, j * 128:(j + 1) * 128], st)
                    h = SCG // 2
                    nc.scalar.copy(
                        srcb[:, g * SCG:g * SCG + h, :], sg[:, :h, :].bitcast(F32)
                    )
                    nc.vector.tensor_copy(
                        srcb[:, g * SCG + h:(g + 1) * SCG, :], sg[:, h:, :].bitcast(F32)
                    )
                    nb, ng = (b, g + 2) if g + 2 < SG else (b + 1, g + 2 - SG)
                    if nb < B:
                        src_g.append(load_src_group(nb, ng))
                state["src_next"] = src_g
                return dict(inpT=inpT, inpTb=inpTb, xT=xT, srcT=srcT, srcb=srcb)

            P = prep(0)
            for b in range(B):
                cur = P
                xT, srcT, srcb, inpTb = cur["xT"], cur["srcT"], cur["srcb"], cur["inpTb"]

                # ---- per t-chunk: mm2 (fp32r) + softmax + attn out + attn^T ----
                # attn^T transposes for chunk i are emitted after mm2(i+1) so the
                # PE has work while softmax(i+1) drains the logits banks.
                attnT = attnTp.tile([128, SC, T], BF16, tag="attnT")
                attn_tiles = []

                def emit_attnT(i, attn_sb):
                    for gg in range(SC // 4):
                        st = tpsp.tile([128, 4, 128], F32R, tag="tps")
                        for q in range(4):
                            j = gg * 4 + q
                            pe_transpose(st[:, q, :], attn_sb[:, j * 128:(j + 1) * 128], True)
                        copy_out(attnT[:, gg * 4:gg * 4 + 4, i * 128:(i + 1) * 128], st)

                NJ = S // 512
                for i in range(TC):
                    # flash-style: each slice exps against ITS OWN max so the
                    # logits slot frees immediately after its chain; the
                    # exp(m_j - M)/Z corrections fold into the final scale.
                    nm = smalls.tile([128, NJ], F32, tag="negpm")  # -m_j
                    sumexp4 = smalls.tile([128, NJ], F32, tag="sumexp")
                    attn_sb = attnp.tile([128, S], F32R, tag="attn")
                    for j in range(NJ):
                        lg = logp.tile([128, 512], F32, tag="logits")
                        for k in range(DC):
                            nc.tensor.matmul(
                                lg,
                                xT[:, k, i * 128:(i + 1) * 128],
                                srcT[:, k, j * 512:(j + 1) * 512],
                                start=(k == 0),
                                stop=(k == DC - 1),
                            )
                        nc.vector.tensor_reduce(
                            nm[:, j:j + 1], lg, axis=AX.X,
                            op=mybir.AluOpType.max, negate=True,
                        )
                        nc.scalar.activation(
                            attn_sb[:, j * 512:(j + 1) * 512],
                            lg,
                            AF.Exp,
                            bias=nm[:, j:j + 1],
                            accum_out=sumexp4[:, j:j + 1],
                        )
                    # NM = -M = min_j nm_j; dl_j = nm_j - NM = M - m_j >= 0
                    NM = smalls.tile([128, 1], F32, tag="NM")
                    nc.vector.tensor_reduce(NM, nm, axis=AX.X, op=mybir.AluOpType.min)
                    dl = smalls.tile([128, NJ], F32, tag="dl")
                    nc.vector.tensor_scalar_sub(dl, nm, NM[:, 0:1])
                    fac = smalls.tile([128, NJ], F32, tag="fac")
                    nc.scalar.activation(fac, dl, AF.Exp, scale=-1.0)  # exp(m_j - M)
                    wsum = smalls.tile([128, NJ], F32, tag="wsum")
                    nc.vector.tensor_mul(wsum, sumexp4, fac)
                    sumtot = smalls.tile([128, 1], F32, tag="sumtot")
                    nc.vector.tensor_reduce(
                        sumtot, wsum, axis=AX.X, op=mybir.AluOpType.add
                    )
                    inv = smalls.tile([128, 1], F32, tag="inv")
                    nc.vector.reciprocal(inv, sumtot)
                    ff = smalls.tile([128, NJ], F32, tag="ff")
                    nc.vector.tensor_scalar_mul(ff, fac, inv[:, 0:1])
                    for j in range(NJ):
                        nc.vector.tensor_scalar_mul(
                            attn_sb[:, j * 512:(j + 1) * 512],
                            attn_sb[:, j * 512:(j + 1) * 512],
                            ff[:, j:j + 1],
                        )
                    nc.gpsimd.dma_start(
                        out=attn_d[b, i * 128:(i + 1) * 128, :],
                        in_=attn_sb.bitcast(F32),
                    )
                    attn_tiles.append(attn_sb)
                    if i > 0:
                        emit_attnT(i - 1, attn_tiles[i - 1])

                # next batch's prep covers the softmax/attnT tail on the PE
                if b + 1 < B:
                    P = prep(b + 1)
                emit_attnT(TC - 1, attn_tiles[TC - 1])

                # ---- mm3: mix^T[e,t] (bf16) ----
                mixT = mixTp.tile([128, DC, T], BF16, tag="mixT")
                for m in range(DC):
                    ps = mmoutp.tile([128, T], F32, tag="mmout")
                    for j in range(SC):
                        nc.tensor.matmul(
                            ps,
                            srcb[:, j, m * 128:(m + 1) * 128],
                            attnT[:, j, :],
                            start=(j == 0),
                            stop=(j == SC - 1),
                        )
                    copy_out(mixT[:, m, :], ps)

                # ---- mm4: out[t,o] = tanh(comb^T.T @ W_out^T) (bf16) ----
                for i in range(TC):
                    ps = mmoutp.tile([128, D], F32, tag="mmout")
                    for k in range(CC):
                        lhsT = (
                            mixT[:, k, i * 128:(i + 1) * 128]
                            if k < DC
                            else inpTb[:, k - DC, i * 128:(i + 1) * 128]
                        )
                        nc.tensor.matmul(
                            ps, lhsT, woutT[:, k, :], start=(k == 0), stop=(k == CC - 1)
                        )
                    ot = outp.tile([128, D], F32, tag="out")
                    nc.scalar.activation(ot, ps, AF.Tanh)
                    nc.gpsimd.dma_start(out=out_d[b, i * 128:(i + 1) * 128, :], in_=ot)

    nc.compile()
    return nc


_NC_CACHE = {}


def get_nc(B=4, T=512, S=2048, D=512):
    key = (B, T, S, D)
    if key not in _NC_CACHE:
        _NC_CACHE[key] = build_nc(B, T, S, D)
    return _NC_CACHE[key]


def kernel(input, source_hids, W_in, W_out, trace=False, trace_kwargs=None):
    from concourse.bass_utils import run_bass_kernel_spmd

    input = np.ascontiguousarray(input, dtype=np.float32)
    source_hids = np.ascontiguousarray(source_hids, dtype=np.float32)
    W_in = np.ascontiguousarray(W_in, dtype=np.float32)
    W_out = np.ascontiguousarray(W_out, dtype=np.float32)

    NCORE = 8
    Bfull = input.shape[0]
    bpc = Bfull // NCORE
    nc = get_nc(B=bpc, T=input.shape[1], S=source_hids.shape[1], D=input.shape[2])

    in_maps = [
        {
            "input": input[c * bpc:(c + 1) * bpc],
            "source_hids": source_hids[c * bpc:(c + 1) * bpc],
            "W_in": W_in,
            "W_out": W_out,
        }
        for c in range(NCORE)
    ]
    res = run_bass_kernel_spmd(
        nc, in_maps, core_ids=list(range(NCORE)), trace=trace, **(trace_kwargs or {})
    )
    output = np.concatenate([res.results[c]["output"] for c in range(NCORE)], axis=0)
    attn = np.concatenate([res.results[c]["attn"] for c in range(NCORE)], axis=0)
    if trace:
        kernel.last_exec_time_ns = res.exec_time_ns
        kernel.last_results = res
    return output, attn


# revision 66
# speedup vs baseline: 1.1031x; 1.0149x over previous
"""Trainium2 Bass kernel for nn_Attention (B=32,T=512,S=2048,D=512), 8-core data parallel.

Per core: 4 batch elements. Per batch:
  x^T   = W_in^T.T @ input^T              (mm1, float32r)
  logits= x^T.T @ src^T                   (mm2, float32r)   [t,s]
  attn  = softmax(logits)  (exact, fp32)  -> DRAM output
  mix^T = src.T @ attn^T                  (mm3, bf16)
  out   = tanh([mix^T;input^T].T @ W_out^T) (mm4, bf16) -> DRAM output

Transposes on PE via identity-matmul into PSUM staging [128,4,128] groups,
copied out (with dtype rounding/cast) on DVE/ACT. fp32r operand chains are
typed float32r end-to-end (walrus requires producers to emit fp32r).
"""
import sys

sys.path.insert(0, "/opt/trn_rl_repo")

import numpy as np

import concourse.bass as bass  # noqa: F401
import concourse.tile as tile
from concourse import bacc, mybir
from concourse.masks import make_identity

F32 = mybir.dt.float32
F32R = mybir.dt.float32r
BF16 = mybir.dt.bfloat16
AX = mybir.AxisListType
AF = mybir.ActivationFunctionType


def build_nc(B=4, T=512, S=2048, D=512):
    TC, SC, DC = T // 128, S // 128, D // 128
    C = 2 * D
    CC = C // 128
    SG = 4            # source_hids DMA'd in SG groups
    SCG = SC // SG

    import concourse.tile_utils as tile_utils
    tile_utils.max_sbuf_usage = 206 * 1024

    nc = bacc.Bacc(None, target_bir_lowering=False, debug=False)
    inp_d = nc.declare_dram_parameter("input", [B, T, D], F32R, isOutput=False)
    src_d = nc.declare_dram_parameter("source_hids", [B, S, D], F32R, isOutput=False)
    win_d = nc.declare_dram_parameter("W_in", [D, D], F32R, isOutput=False)
    wout_d = nc.declare_dram_parameter("W_out", [D, C], F32, isOutput=False)
    out_d = nc.declare_dram_parameter("output", [B, T, D], F32, isOutput=True)
    attn_d = nc.declare_dram_parameter("attn", [B, T, S], F32, isOutput=True)

    from contextlib import ExitStack

    with ExitStack() as ctx:
        tc = ctx.enter_context(tile.TileContext(nc))
        if True:
            pool = lambda name, bufs, **kw: ctx.enter_context(
                tc.tile_pool(name=name, bufs=bufs, **kw)
            )
            consts = pool("consts", 1)
            wkeep = pool("wkeep", 1)
            inputp = pool("inputp", 2)
            inputTp = pool("inputTp", 1)
            inputTbp = pool("inputTbp", 2)
            srcgp = pool("srcgp", 3)
            srcbp = pool("srcbp", 2)
            srcTp = pool("srcTp", 1)
            xTp = pool("xTp", 2)
            attnp = pool("attnp", 3)
            attnTp = pool("attnTp", 1)
            mixTp = pool("mixTp", 2)
            outp = pool("outp", 3)
            smalls = pool("smalls", 3)
            logp = pool("logp", 4, space="PSUM")
            mmoutp = pool("mmoutp", 2, space="PSUM")
            tpsp = pool("tpsp", 2, space="PSUM")

            ident = consts.tile([128, 128], F32)
            make_identity(nc, ident)
            ident_r = consts.tile([128, 128], F32R)
            nc.vector.tensor_copy(ident_r, ident)
            def pe_transpose(dst_psum, src_sbuf, rounded):
                idn = ident_r if rounded else ident
                nc.tensor.matmul(dst_psum, src_sbuf, idn, is_transpose=True)

            copy_flip = [0]

            def copy_out(dst, src_psum, engine=None):
                if engine is None:
                    engine = "act" if copy_flip[0] % 2 == 0 else "dve"
                    copy_flip[0] += 1
                if engine == "act":
                    nc.scalar.copy(dst, src_psum)
                else:
                    nc.vector.tensor_copy(dst, src_psum)

            # ---- batch-0 loads first so they overlap weight prep ----
            def load_input(b):
                t = inputp.tile([128, TC, D], F32R, tag="inp")
                nc.sync.dma_start(
                    out=t, in_=inp_d[b].rearrange("(c p) d -> p c d", p=128)
                )
                return t

            def load_src_group(b, g):
                sg = srcgp.tile([128, SCG, D], F32R, tag="srcg")
                nc.sync.dma_start(
                    out=sg,
                    in_=src_d[b, g * SCG * 128:(g + 1) * SCG * 128, :].rearrange(
                        "(c p) d -> p c d", p=128
                    ),
                )
                return sg

            inp_next = load_input(0)

            # ---- weights (once; staged through the srcgp pool) ----
            # W_in [e,d] -> W_inT [d-in, DC, e]  (f32r, feeds mm1 lhsT)
            win_s = srcgp.tile([128, DC, D], F32R, tag="srcg")
            nc.scalar.dma_start(out=win_s, in_=win_d.rearrange("(c p) d -> p c d", p=128))
            winT = wkeep.tile([128, DC, D], F32R, tag="winT")
            for ec in range(DC):
                st = tpsp.tile([128, 4, 128], F32R, tag="tps")
                for dc in range(DC):
                    pe_transpose(st[:, dc, :], win_s[:, ec, dc * 128:(dc + 1) * 128], True)
                copy_out(winT[:, :, ec * 128:(ec + 1) * 128], st)
            state = {
                "inp_next": inp_next,
                "src_next": [load_src_group(0, 0), load_src_group(0, 1)],
            }

            def prep(b):
                """input^T + mm1 + src^T + casts for batch b. Emitted between
                mm2(b-1) and mm3(b-1) so the PE covers softmax tails."""
                inp_t = state["inp_next"]
                src_g = state["src_next"]

                inpT = inputTp.tile([128, DC, T], F32R, tag="inpT")
                for c in range(TC):
                    st = tpsp.tile([128, 4, 128], F32R, tag="tps")
                    for k in range(DC):
                        pe_transpose(st[:, k, :], inp_t[:, c, k * 128:(k + 1) * 128], True)
                    copy_out(inpT[:, :, c * 128:(c + 1) * 128], st)
                inpTb = inputTbp.tile([128, DC, T], BF16, tag="inpTb")
                nc.vector.tensor_copy(inpTb, inpT.bitcast(F32))
                if b + 1 < B:
                    state["inp_next"] = load_input(b + 1)

                xT = xTp.tile([128, DC, T], F32R, tag="xT")
                for m in range(DC):
                    ps = mmoutp.tile([128, T], F32, tag="mmout")
                    for k in range(DC):
                        nc.tensor.matmul(
                            ps,
                            winT[:, k, m * 128:(m + 1) * 128],
                            inpT[:, k, :],
                            start=(k == 0),
                            stop=(k == DC - 1),
                        )
                    copy_out(xT[:, m, :], ps)

                srcT = srcTp.tile([128, DC, S], F32R, tag="srcT")
                srcb = srcbp.tile([128, SC, D], BF16, tag="srcb")
                for g in range(SG):
                    sg = src_g.pop(0)
                    for jj in range(SCG):
                        j = g * SCG + jj
                        st = tpsp.tile([128, 4, 128], F32R, tag="tps")
                        for m in range(DC):
                            pe_transpose(
                                st[:, m, :], sg[:, jj, m * 128:(m + 1) * 128], True
                            )
                        copy_out(srcT[:, :, j * 128:(j + 1) * 128], st)
                    h = SCG // 2
                    nc.scalar.copy(
                        srcb[:, g * SCG:g * SCG + h, :], sg[:, :h, :].bitcast(F32)
                    )
                    nc.vector.tensor_copy(
                        srcb[:, g * SCG + h:(g + 1) * SCG, :], sg[:, h:, :].bitcast(F32)
                    )
                    nb, ng = (b, g + 2) if g + 2 < SG else (b + 1, g + 2 - SG)
                    if nb < B:
                        src_g.append(load_src_group(nb, ng))
                state["src_next"] = src_g
                return dict(inpT=inpT, inpTb=inpTb, xT=xT, srcT=srcT, srcb=srcb)

            P = prep(0)
            # W_out [o,c] -> W_outT [c-in, CC, o] bf16, staged in two halves
            woutT = wkeep.tile([128, CC, D], BF16, tag="woutT")
            for h in range(2):
                wout_s = srcgp.tile([128, DC, D], F32, tag="srcg")
                nc.scalar.dma_start(
                    out=wout_s,
                    in_=wout_d[:, h * D:(h + 1) * D].rearrange("(c p) d -> p c d", p=128),
                )
                for oc in range(DC):
                    st = tpsp.tile([128, 4, 128], F32, tag="tps")
                    for q in range(4):
                        pe_transpose(st[:, q, :], wout_s[:, oc, q * 128:(q + 1) * 128], False)
                    copy_out(woutT[:, 4 * h:4 * h + 4, oc * 128:(oc + 1) * 128], st)

            for b in range(B):
                cur = P
                xT, srcT, srcb, inpTb = cur["xT"], cur["srcT"], cur["srcb"], cur["inpTb"]

                # ---- per t-chunk: mm2 (fp32r) + softmax + attn out + attn^T ----
                # attn^T transposes for chunk i are emitted after mm2(i+1) so the
                # PE has work while softmax(i+1) drains the logits banks.
                attnT = attnTp.tile([128, SC, T], BF16, tag="attnT")
                attn_tiles = []

                def emit_attnT(i, attn_sb):
                    for gg in range(SC // 4):
                        st = tpsp.tile([128, 4, 128], F32R, tag="tps")
                        for q in range(4):
                            j = gg * 4 + q
                            pe_transpose(st[:, q, :], attn_sb[:, j * 128:(j + 1) * 128], True)
                        copy_out(attnT[:, gg * 4:gg * 4 + 4, i * 128:(i + 1) * 128], st)

                NJ = S // 512
                for i in range(TC):
                    # flash-style: each slice exps against ITS OWN max so the
                    # logits slot frees immediately after its chain; the
                    # exp(m_j - M)/Z corrections fold into the final scale.
                    nm = smalls.tile([128, NJ], F32, tag="negpm")  # -m_j
                    sumexp4 = smalls.tile([128, NJ], F32, tag="sumexp")
                    attn_sb = attnp.tile([128, S], F32R, tag="attn")
                    for j in range(NJ):
                        lg = logp.tile([128, 512], F32, tag="logits")
                        for k in range(DC):
                            nc.tensor.matmul(
                                lg,
                                xT[:, k, i * 128:(i + 1) * 128],
                                srcT[:, k, j * 512:(j + 1) * 512],
                                start=(k == 0),
                                stop=(k == DC - 1),
                            )
                        nc.vector.tensor_reduce(
                            nm[:, j:j + 1], lg, axis=AX.X,
                            op=mybir.AluOpType.max, negate=True,
                        )
                        nc.scalar.activation(
                            attn_sb[:, j * 512:(j + 1) * 512],
                            lg,
                            AF.Exp,
                            bias=nm[:, j:j + 1],
                            accum_out=sumexp4[:, j:j + 1],
                        )
                    # NM = -M = min_j nm_j; dl_j = nm_j - NM = M - m_j >= 0
                    NM = smalls.tile([128, 1], F32, tag="NM")
                    nc.vector.tensor_reduce(NM, nm, axis=AX.X, op=mybir.AluOpType.min)
                    dl = smalls.tile([128, NJ], F32, tag="dl")
                    nc.vector.tensor_scalar_sub(dl, nm, NM[:, 0:1])
                    fac = smalls.tile([128, NJ], F32, tag="fac")
                    nc.scalar.activation(fac, dl, AF.Exp, scale=-1.0)  # exp(m_j - M)
                    wsum = smalls.tile([128, NJ], F32, tag="wsum")
                    nc.vector.tensor_mul(wsum, sumexp4, fac)
                    sumtot = smalls.tile([128, 1], F32, tag="sumtot")
                    nc.vector.tensor_reduce(
                        sumtot, wsum, axis=AX.X, op=mybir.AluOpType.add
                    )
                    inv = smalls.tile([128, 1], F32, tag="inv")
                    nc.vector.reciprocal(inv, sumtot)
                    ff = smalls.tile([128, NJ], F32, tag="ff")
                    nc.vector.tensor_scalar_mul(ff, fac, inv[:, 0:1])
                    for j in range(NJ):
                        nc.vector.tensor_scalar_mul(
                            attn_sb[:, j * 512:(j + 1) * 512],
                            attn_sb[:, j * 512:(j + 1) * 512],
                            ff[:, j:j + 1],
                        )
                    nc.gpsimd.dma_start(
                        out=attn_d[b, i * 128:(i + 1) * 128, :],
                        in_=attn_sb.bitcast(F32),
                    )
                    attn_tiles.append(attn_sb)
                    if i > 0:
                        emit_attnT(i - 1, attn_tiles[i - 1])

                # next batch's prep covers the softmax/attnT tail on the PE
                if b + 1 < B:
                    P = prep(b + 1)
                emit_attnT(TC - 1, attn_tiles[TC - 1])

                # ---- mm3: mix^T[e,t] (bf16) ----
                mixT = mixTp.tile([128, DC, T], BF16, tag="mixT")
                for m in range(DC):
                    ps = mmoutp.tile([128, T], F32, tag="mmout")
                    for j in range(SC):
                        nc.tensor.matmul(
                            ps,
                            srcb[:, j, m * 128:(m + 1) * 128],
                            attnT[:, j, :],
                            start=(j == 0),
                            stop=(j == SC - 1),
                        )
                    copy_out(mixT[:, m, :], ps)

                # ---- mm4: out[t,o] = tanh(comb^T.T @ W_out^T) (bf16) ----
                for i in range(TC):
                    ps = mmoutp.tile([128, D], F32, tag="mmout")
                    for k in range(CC):
                        lhsT = (
                            mixT[:, k, i * 128:(i + 1) * 128]
                            if k < DC
                            else inpTb[:, k - DC, i * 128:(i + 1) * 128]
                        )
                        nc.tensor.matmul(
                            ps, lhsT, woutT[:, k, :], start=(k == 0), stop=(k == CC - 1)
                        )
                    ot = outp.tile([128, D], F32, tag="out")
                    nc.scalar.activation(ot, ps, AF.Tanh)
                    nc.gpsimd.dma_start(out=out_d[b, i * 128:(i + 1) * 128, :], in_=ot)

    nc.compile()
    return nc


_NC_CACHE = {}


def get_nc(B=4, T=512, S=2048, D=512):
    key = (B, T, S, D)
    if key not in _NC_CACHE:
        _NC_CACHE[key] = build_nc(B, T, S, D)
    return _NC_CACHE[key]


def kernel(input, source_hids, W_in, W_out, trace=False, trace_kwargs=None):
    from concourse.bass_utils import run_bass_kernel_spmd

    input = np.ascontiguousarray(input, dtype=np.float32)
    source_hids = np.ascontiguousarray(source_hids, dtype=np.float32)
    W_in = np.ascontiguousarray(W_in, dtype=np.float32)
    W_out = np.ascontiguousarray(W_out, dtype=np.float32)

    NCORE = 8
    Bfull = input.shape[0]
    bpc = Bfull // NCORE
    nc = get_nc(B=bpc, T=input.shape[1], S=source_hids.shape[1], D=input.shape[2])

    in_maps = [
        {
            "input": input[c * bpc:(c + 1) * bpc],
            "source_hids": source_hids[c * bpc:(c + 1) * bpc],
            "W_in": W_in,
            "W_out": W_out,
        }
        for c in range(NCORE)
    ]
    res = run_bass_kernel_spmd(
        nc, in_maps, core_ids=list(range(NCORE)), trace=trace, **(trace_kwargs or {})
    )
    output = np.concatenate([res.results[c]["output"] for c in range(NCORE)], axis=0)
    attn = np.concatenate([res.results[c]["attn"] for c in range(NCORE)], axis=0)
    if trace:
        kernel.last_exec_time_ns = res.exec_time_ns
        kernel.last_results = res
    return output, attn
